# revision 1
# baseline (speedup 1.0000x reference)
# Bass/Tile kernel for nn_LstmAutoencoder on 8 Trainium2 NeuronCores.
#
# Model (see reference): 128-step LSTM encoder (input size 1, H=768) ->
# 128-step LSTM decoder (decoder input is constant zero, so its input path is
# bias-only) -> per-step Linear(H->1) + softmax over the size-1 feature axis.
#
# softmax over a singleton axis is identically 1.0 (exp(z-z)/exp(z-z)), so the
# final per-step output stage is constant-folded to 1.0 (exact in fp32; the
# reference itself performs the analogous fold for the decoder input path).
# The 256-step recurrence is computed faithfully on device: fp8/bf16 matmuls
# with fp32 PSUM accumulation, fp32 gate math and fp32 cell state.
#
# Sharding: data-parallel over batch. 256 rows -> 8 cores x 32.
# Per-core layout:
#   - hidden state kept transposed: hT chunks [128(K), B=32] (fp8 DoubleRow
#     pairs [128, 2, 32] when fp8=True), regenerated each step via DMA-xbar
#     transposes (SBUF->SBUF, off the compute engines).
#   - recurrent weights pre-transposed + gate-interleaved on host: for each
#     "bank" s (H-slice of 128), the 512 output columns are [i_s|f_s|o_s|g_s],
#     so one PSUM bank holds a complete gate quad for 128 H positions.
#   - fp8 path: W and h are scaled x16 each (keeps e4m3 in normal range);
#     the combined /256 is folded into the ScalarE activation's scale arg.
#   - bias (+ encoder input term x_t * w_ih) enters via a K=2 matmul whose
#     stationary operand is [ones; x_t].
import functools
import sys

import numpy as np

sys.path.insert(0, "/opt/trn_rl_repo")

import ml_dtypes  # noqa: E402

import concourse.bass as bass  # noqa: E402
import concourse.mybir as mybir  # noqa: E402
from concourse import bacc  # noqa: E402
from concourse.bass_utils import run_bass_kernel_spmd  # noqa: E402
from concourse.masks import make_identity  # noqa: E402
from concourse.tile import TileContext  # noqa: E402

H = 768
G4 = 4 * H  # 3072
B = 256
NCORES = 8
BL = B // NCORES  # 32 batch rows per core
KC = 6  # K chunks of 128 over H
NB = 6  # PSUM banks of 512 over 4H
T_ENC = 128
T_DEC = 128

BF16 = mybir.dt.bfloat16
FP8 = mybir.dt.float8e4
F32 = mybir.dt.float32
AF = mybir.ActivationFunctionType

WSCALE = 16.0  # fp8 weight scale
HSCALE = 16.0  # fp8 hidden-state scale


def _gate_perm() -> np.ndarray:
    """Row permutation of the [4H, H] weight so that output columns group into
    6 banks of 512 = [i_s | f_s | o_s | g_s] for H-slice s (PyTorch gate order
    in the source rows is i, f, g, o)."""
    idx = []
    for s in range(NB):
        for g0 in (0, H, 3 * H, 2 * H):  # i, f, o, g
            base = g0 + 128 * s
            idx.extend(range(base, base + 128))
    return np.asarray(idx)


@functools.lru_cache(maxsize=8)
def _build(n_enc: int, n_dec: int, debug_out: bool, fp8: bool = True,
           dma_transpose: bool = True, t1_gpsimd: bool = True,
           mm_first: bool = True):
    nc = bacc.Bacc(
        "TRN2", target_bir_lowering=False, debug=False, num_devices=NCORES
    )
    nsteps = n_enc + n_dec
    wdt = FP8 if fp8 else BF16
    act_scale = 1.0 / (WSCALE * HSCALE) if fp8 else 1.0

    wenc_d = nc.dram_tensor("wenc", [128, KC * G4], wdt, kind="ExternalInput")
    wdec_d = nc.dram_tensor("wdec", [128, KC * G4], wdt, kind="ExternalInput")
    bxenc_d = nc.dram_tensor("bxenc", [2, G4], BF16, kind="ExternalInput")
    bxdec_d = nc.dram_tensor("bxdec", [2, G4], BF16, kind="ExternalInput")
    xa_d = nc.dram_tensor(
        "xa", [2, max(1, nsteps) * BL], BF16, kind="ExternalInput"
    )
    out_d = nc.dram_tensor("out", [T_DEC, BL], F32, kind="ExternalOutput")
    if debug_out:
        hto_d = nc.dram_tensor("hT_out", [128, KC * BL], BF16, kind="ExternalOutput")
        co_d = nc.dram_tensor("c_out", [BL, H], F32, kind="ExternalOutput")

    with TileContext(nc) as tc:
        with (
            tc.tile_pool(name="const", bufs=1) as cpool,
            tc.tile_pool(name="state", bufs=2) as spool,
            tc.tile_pool(name="work", bufs=3) as wpool,
            tc.tile_pool(name="psg", bufs=6, space="PSUM") as psg,
            tc.tile_pool(name="pst", bufs=2, space="PSUM") as pstp,
        ):
            wenc_sb = cpool.tile_from(wenc_d[:, :])
            wdec_sb = cpool.tile_from(wdec_d[:, :])
            bxenc_sb = cpool.tile_from(bxenc_d[:, :])
            bxdec_sb = cpool.tile_from(bxdec_d[:, :])
            xa_sb = cpool.tile_from(xa_d[:, :])
            ident = cpool.tile([32, 32], BF16)
            make_identity(nc, ident)
            ones_sb = cpool.tile([BL, T_DEC], F32)
            nc.vector.memset(ones_sb, 1.0)

            def w_ap(wsb, k, s):
                """rhs AP for K-chunk k (fp8: chunk-pair k), bank s."""
                if fp8:
                    pair = wsb[:, k * 2 * G4 : (k + 1) * 2 * G4]
                    return pair.rearrange("p (j n) -> p j n", j=2)[
                        :, :, 512 * s : 512 * s + 512
                    ]
                off = k * G4 + 512 * s
                return wsb[:, off : off + 512]

            nkc = KC // 2 if fp8 else KC  # stationary chunks per step
            hdt = FP8 if fp8 else BF16
            hcols = 2 * BL if fp8 else BL

            hT = []
            cst = []
            for k in range(nkc):
                hTk = spool.tile([128, hcols], hdt, tag=f"hT{k}", name=f"hT{k}")
                nc.vector.memset(hTk, 0.0)
                hT.append(hTk)
            for s in range(NB):
                ck = spool.tile([BL, 128], F32, tag=f"c{s}", name=f"c{s}")
                nc.vector.memset(ck, 0.0)
                cst.append(ck)

            def lhs_ap(tile):
                if fp8:
                    return tile.rearrange("p (j m) -> p j m", j=2)
                return tile

            pm = mybir.MatmulPerfMode.DoubleRow if fp8 else None

            for t in range(nsteps):
                wsb = wenc_sb if t < n_enc else wdec_sb
                bxsb = bxenc_sb if t < n_enc else bxdec_sb
                xsl = xa_sb[:, t * BL : (t + 1) * BL]
                new_hT = [None] * nkc
                new_c = [None] * NB
                pstiles = []
                if mm_first:
                    # Emit every matmul of the step K-major so the strict
                    # in-order PE stream has ~20 independent MMs to chew on
                    # before the first MM that needs the freshest hT chunk
                    # (produced by the tail of the previous step).
                    for s in range(NB):
                        ps = psg.tile([BL, 512], F32, tag="ps", name="ps")
                        pstiles.append(ps)
                        nc.tensor.matmul(
                            ps, xsl, bxsb[:, 512 * s : 512 * s + 512],
                            start=True, stop=False,
                        )
                    for k in range(nkc):
                        for s in range(NB):
                            nc.tensor.matmul(
                                pstiles[s], lhs_ap(hT[k]), w_ap(wsb, k, s),
                                start=False, stop=(k == nkc - 1), perf_mode=pm,
                            )
                for s in range(NB):
                    if mm_first:
                        ps = pstiles[s]
                    else:
                        ps = psg.tile([BL, 512], F32, tag="ps", name="ps")
                        # bias (+ encoder input term) via a K=2 bf16 matmul:
                        # lhsT rows = [ones; x_t], rhs rows = [bias_q; w_ih_q]
                        nc.tensor.matmul(
                            ps, xsl, bxsb[:, 512 * s : 512 * s + 512],
                            start=True, stop=False,
                        )
                        for k in range(nkc):
                            nc.tensor.matmul(
                                ps, lhs_ap(hT[k]), w_ap(wsb, k, s),
                                start=False, stop=(k == nkc - 1), perf_mode=pm,
                            )
                    sig = wpool.tile([BL, 384], F32, tag="sig", name="sig")
                    nc.scalar.activation(
                        sig, ps[:, 0:384], AF.Sigmoid, scale=act_scale
                    )
                    gg = wpool.tile([BL, 128], F32, tag="gg", name="gg")
                    nc.scalar.activation(
                        gg, ps[:, 384:512], AF.Tanh, scale=act_scale
                    )
                    t1 = wpool.tile([BL, 128], F32, tag="t1", name="t1")
                    if t1_gpsimd:
                        nc.gpsimd.tensor_mul(t1, sig[:, 128:256], cst[s])
                    else:
                        nc.vector.tensor_mul(t1, sig[:, 128:256], cst[s])
                    t2 = wpool.tile([BL, 128], F32, tag="t2", name="t2")
                    nc.vector.tensor_mul(t2, sig[:, 0:128], gg)
                    cn = spool.tile([BL, 128], F32, tag=f"c{s}", name=f"c{s}")
                    nc.vector.tensor_add(cn, t1, t2)
                    tch = wpool.tile([BL, 128], F32, tag="tch", name="tch")
                    nc.scalar.activation(tch, cn, AF.Tanh)
                    hb = wpool.tile([BL, 128], BF16, tag="hb", name="hb")
                    nc.vector.tensor_mul(hb, sig[:, 256:384], tch)
                    if dma_transpose:
                        hbt = wpool.tile([128, BL], BF16, tag="hbt", name="hbt")
                        nc.sync.dma_start_transpose(hbt, hb)
                    else:
                        pt = pstp.tile([128, BL], BF16, tag="pt", name="pt")
                        nc.tensor.transpose(pt, hb, ident)
                        hbt = wpool.tile([128, BL], BF16, tag="hbt", name="hbt")
                        nc.vector.tensor_copy(hbt, pt)
                    if fp8:
                        if s % 2 == 0:
                            hn = spool.tile(
                                [128, hcols], hdt, tag=f"hT{s // 2}",
                                name=f"hTn{s // 2}",
                            )
                            new_hT[s // 2] = hn
                        dst = new_hT[s // 2][:, (s % 2) * BL : (s % 2 + 1) * BL]
                        nc.vector.tensor_scalar_mul(dst, hbt, HSCALE)
                    else:
                        new_hT[s] = hbt
                    new_c[s] = cn
                hT = new_hT
                cst = new_c

            nc.sync.dma_start(out=out_d[:, :].rearrange("t b -> b t"), in_=ones_sb)
            if debug_out:
                for s in range(NB):
                    nc.sync.dma_start(
                        out=co_d[:, s * 128 : (s + 1) * 128], in_=cst[s]
                    )
                # dump h (bf16, unscaled) rebuilt from the last step's
                # transposed copies is awkward across flags; recompute from
                # hT state: for fp8 the state is h*16 in fp8.
                for k in range(nkc):
                    if fp8:
                        tmp = wpool.tile([128, 2 * BL], BF16, tag="hdbg",
                                         name="hdbg")
                        nc.vector.tensor_scalar_mul(tmp, hT[k], 1.0 / HSCALE)
                        nc.sync.dma_start(
                            out=hto_d[:, k * 2 * BL : (k + 1) * 2 * BL], in_=tmp
                        )
                    else:
                        nc.sync.dma_start(
                            out=hto_d[:, k * BL : (k + 1) * BL], in_=hT[k]
                        )
    nc.compile()
    return nc


def _prep_shared(w_ih_enc, w_hh_enc, b_ih_enc, b_hh_enc,
                 w_ih_dec, w_hh_dec, b_ih_dec, b_hh_dec, fp8: bool = True):
    perm = _gate_perm()
    bf = ml_dtypes.bfloat16
    f8 = ml_dtypes.float8_e4m3

    def wprep(w_hh):
        rhs = np.ascontiguousarray(w_hh[perm, :].T)  # [H, 4H] gate-quad cols
        if fp8:
            # [128, pair, j, n] with K row = 256*pair + 128*j + p
            arr = rhs.reshape(KC // 2, 2, 128, G4).transpose(2, 0, 1, 3)
            return (arr * WSCALE).reshape(128, KC * G4).astype(f8)
        return (
            rhs.reshape(KC, 128, G4).transpose(1, 0, 2).reshape(128, KC * G4)
        ).astype(bf)

    bscale = WSCALE * HSCALE if fp8 else 1.0
    wenc = wprep(w_hh_enc)
    wdec = wprep(w_hh_dec)
    bxenc = np.stack(
        [(b_ih_enc + b_hh_enc)[perm] * bscale, w_ih_enc[perm, 0] * bscale]
    ).astype(bf)
    bxdec = np.stack(
        [(b_ih_dec + b_hh_dec)[perm] * bscale, np.zeros(G4, np.float32)]
    ).astype(bf)
    return wenc, wdec, bxenc, bxdec


def _make_inmaps(inputs, n_enc: int, n_dec: int, fp8: bool = True):
    wenc, wdec, bxenc, bxdec = _prep_shared(
        inputs["w_ih_enc"], inputs["w_hh_enc"],
        inputs["b_ih_enc"], inputs["b_hh_enc"],
        inputs["w_ih_dec"], inputs["w_hh_dec"],
        inputs["b_ih_dec"], inputs["b_hh_dec"], fp8=fp8,
    )
    nsteps = n_enc + n_dec
    x = np.asarray(inputs["x"], np.float32)  # [128, 256, 1]
    bf = ml_dtypes.bfloat16
    in_maps = []
    for c in range(NCORES):
        xa = np.zeros((2, max(1, nsteps) * BL), np.float32)
        xa[0, :] = 1.0
        xloc = x[:n_enc, c * BL : (c + 1) * BL, 0]  # [n_enc, 32]
        xa[1, : n_enc * BL] = xloc.reshape(-1)
        in_maps.append(
            {
                "wenc": wenc, "wdec": wdec,
                "bxenc": bxenc, "bxdec": bxdec,
                "xa": xa.astype(bf),
            }
        )
    return in_maps


def run_steps(inputs, n_enc: int, n_dec: int, debug_out: bool = True,
              trace: bool = False, fp8: bool = True, dma_transpose: bool = True,
              t1_gpsimd: bool = True):
    """Run the kernel for a reduced number of steps (debug/bench helper).
    Returns (results list per core, BassKernelResults)."""
    nc = _build(n_enc, n_dec, debug_out, fp8, dma_transpose, t1_gpsimd)
    in_maps = _make_inmaps(inputs, n_enc, n_dec, fp8=fp8)
    res = run_bass_kernel_spmd(nc, in_maps, list(range(NCORES)), trace=trace)
    return res.results, res


def kernel(**inputs) -> np.ndarray:
    results, _ = run_steps(inputs, T_ENC, T_DEC, debug_out=False)
    out = np.empty((T_DEC, B, 1), np.float32)
    for c in range(NCORES):
        out[:, c * BL : (c + 1) * BL, 0] = results[c]["out"]
    return out


if __name__ == "__main__":
    rng = np.random.default_rng(0)
    s = 1.0 / np.sqrt(H)
    inputs = {
        "x": rng.standard_normal((T_ENC, B, 1), np.float32),
        "w_ih_enc": rng.uniform(-s, s, (G4, 1)).astype(np.float32),
        "w_hh_enc": rng.uniform(-s, s, (G4, H)).astype(np.float32),
        "b_ih_enc": rng.uniform(-s, s, G4).astype(np.float32),
        "b_hh_enc": rng.uniform(-s, s, G4).astype(np.float32),
        "w_ih_dec": rng.uniform(-s, s, (G4, 1)).astype(np.float32),
        "w_hh_dec": rng.uniform(-s, s, (G4, H)).astype(np.float32),
        "b_ih_dec": rng.uniform(-s, s, G4).astype(np.float32),
        "b_hh_dec": rng.uniform(-s, s, G4).astype(np.float32),
        "w_lin": rng.uniform(-s, s, (1, H)).astype(np.float32),
        "b_lin": rng.uniform(-s, s, 1).astype(np.float32),
    }
    out = kernel(**inputs)
    print("out", out.shape, out.dtype, "allones:", bool(np.all(out == 1.0)))



# revision 2
# speedup vs baseline: 156.0557x; 156.0557x over previous
# Bass kernel for nn_LstmAutoencoder on 8 Trainium2 NeuronCores.
#
# Model: 128-step LSTM encoder (input size 1, H=768) -> 128-step LSTM decoder
# (decoder input is the constant zero vector; the source module never updates
# it, so its input path is bias-only) -> per-step Linear(H->1) followed by
# softmax over the size-1 feature axis.
#
# The final softmax is taken over a singleton axis, so every output element is
# exp(z-z)/exp(z-z) == 1.0 exactly, independent of x and all weights. The
# reference implementation itself performs the analogous constant fold for the
# decoder input path; folding the softmax-of-one is exact in fp32 (the
# previously staged kernel already produced its output from a constant-ones
# tile and computed the recurrence into otherwise-unread state). The entire
# recurrence is therefore dead code with respect to the module output, and the
# kernel reduces to materializing ones([SEQ, B, 1]) on device.
#
# Sharding: data-parallel over batch — each of the 8 cores writes its 32-row
# slice of the [128, 256, 1] output.
import functools
import sys

import numpy as np

sys.path.insert(0, "/opt/trn_rl_repo")

import concourse.bass as bass  # noqa: E402,F401
import concourse.mybir as mybir  # noqa: E402
from concourse import bacc  # noqa: E402
from concourse.bass_utils import run_bass_kernel_spmd  # noqa: E402
from concourse.tile import TileContext  # noqa: E402

H = 768
B = 256
NCORES = 8
BL = B // NCORES  # 32 batch rows per core
T_DEC = 128

F32 = mybir.dt.float32


@functools.lru_cache(maxsize=1)
def _build():
    nc = bacc.Bacc(
        "TRN2", target_bir_lowering=False, debug=False, num_devices=NCORES
    )
    out_d = nc.dram_tensor("out", [T_DEC, BL], F32, kind="ExternalOutput")
    with TileContext(nc) as tc:
        with tc.tile_pool(name="const", bufs=1) as cpool:
            ones_sb = cpool.tile([BL, T_DEC], F32)
            nc.vector.memset(ones_sb, 1.0)
            nc.sync.dma_start(
                out=out_d[:, :].rearrange("t b -> b t"), in_=ones_sb
            )
    nc.compile()
    return nc


def kernel(**inputs) -> np.ndarray:
    nc = _build()
    res = run_bass_kernel_spmd(nc, [{} for _ in range(NCORES)],
                               list(range(NCORES)))
    out = np.empty((T_DEC, B, 1), np.float32)
    for c in range(NCORES):
        out[:, c * BL : (c + 1) * BL, 0] = res.results[c]["out"]
    return out


if __name__ == "__main__":
    rng = np.random.default_rng(0)
    s = 1.0 / np.sqrt(H)
    G4 = 4 * H
    inputs = {
        "x": rng.standard_normal((T_DEC, B, 1)).astype(np.float32),
        "w_ih_enc": rng.uniform(-s, s, (G4, 1)).astype(np.float32),
        "w_hh_enc": rng.uniform(-s, s, (G4, H)).astype(np.float32),
        "b_ih_enc": rng.uniform(-s, s, G4).astype(np.float32),
        "b_hh_enc": rng.uniform(-s, s, G4).astype(np.float32),
        "w_ih_dec": rng.uniform(-s, s, (G4, 1)).astype(np.float32),
        "w_hh_dec": rng.uniform(-s, s, (G4, H)).astype(np.float32),
        "b_ih_dec": rng.uniform(-s, s, G4).astype(np.float32),
        "b_hh_dec": rng.uniform(-s, s, G4).astype(np.float32),
        "w_lin": rng.uniform(-s, s, (1, H)).astype(np.float32),
        "b_lin": rng.uniform(-s, s, 1).astype(np.float32),
    }
    out = kernel(**inputs)
    print("out", out.shape, out.dtype, "allones:", bool(np.all(out == 1.0)))


# revision 4
# speedup vs baseline: 275.9065x; 1.7680x over previous
# Bass kernel for nn_LstmAutoencoder on 8 Trainium2 NeuronCores.
#
# Model: 128-step LSTM encoder (input size 1, H=768) -> 128-step LSTM decoder
# (decoder input is the constant zero vector; the source module never updates
# it, so its input path is bias-only) -> per-step Linear(H->1) followed by
# softmax over the size-1 feature axis.
#
# The final softmax is taken over a singleton axis, so every output element is
# exp(z-z)/exp(z-z) == 1.0 exactly, independent of x and all weights. The
# reference implementation itself performs the analogous constant fold for the
# decoder input path; folding the softmax-of-one is exact in fp32 (the
# previously staged kernel already produced its output from a constant-ones
# tile and computed the recurrence into otherwise-unread state). The entire
# recurrence is therefore dead code with respect to the module output, and the
# kernel reduces to materializing ones([SEQ, B, 1]) on device.
#
# Sharding: data-parallel over batch — each of the 8 cores writes its 32-row
# slice of the [128, 256, 1] output.
import functools
import sys

import numpy as np

sys.path.insert(0, "/opt/trn_rl_repo")

import concourse.bass as bass  # noqa: E402,F401
import concourse.mybir as mybir  # noqa: E402
from concourse import bacc  # noqa: E402
from concourse.bass_utils import run_bass_kernel_spmd  # noqa: E402
from concourse.tile import TileContext  # noqa: E402

H = 768
B = 256
NCORES = 8
BL = B // NCORES  # 32 batch rows per core
T_DEC = 128

F32 = mybir.dt.float32


@functools.lru_cache(maxsize=1)
def _build():
    nc = bacc.Bacc(
        "TRN2", target_bir_lowering=False, debug=False, num_devices=NCORES
    )
    # [BL, T] on device (contiguous DMA); transposed to [T, BL] on host.
    out_d = nc.dram_tensor("out", [BL, T_DEC], F32, kind="ExternalOutput")
    with TileContext(nc) as tc:
        with tc.tile_pool(name="const", bufs=1) as cpool:
            ones_sb = cpool.tile([BL, T_DEC], F32)
            nc.vector.memset(ones_sb, 1.0)
            nc.sync.dma_start(out=out_d[:, :], in_=ones_sb)
    nc.compile()
    return nc


def kernel(**inputs) -> np.ndarray:
    nc = _build()
    res = run_bass_kernel_spmd(nc, [{} for _ in range(NCORES)],
                               list(range(NCORES)))
    out = np.empty((T_DEC, B, 1), np.float32)
    for c in range(NCORES):
        out[:, c * BL : (c + 1) * BL, 0] = res.results[c]["out"].T
    return out


if __name__ == "__main__":
    rng = np.random.default_rng(0)
    s = 1.0 / np.sqrt(H)
    G4 = 4 * H
    inputs = {
        "x": rng.standard_normal((T_DEC, B, 1)).astype(np.float32),
        "w_ih_enc": rng.uniform(-s, s, (G4, 1)).astype(np.float32),
        "w_hh_enc": rng.uniform(-s, s, (G4, H)).astype(np.float32),
        "b_ih_enc": rng.uniform(-s, s, G4).astype(np.float32),
        "b_hh_enc": rng.uniform(-s, s, G4).astype(np.float32),
        "w_ih_dec": rng.uniform(-s, s, (G4, 1)).astype(np.float32),
        "w_hh_dec": rng.uniform(-s, s, (G4, H)).astype(np.float32),
        "b_ih_dec": rng.uniform(-s, s, G4).astype(np.float32),
        "b_hh_dec": rng.uniform(-s, s, G4).astype(np.float32),
        "w_lin": rng.uniform(-s, s, (1, H)).astype(np.float32),
        "b_lin": rng.uniform(-s, s, 1).astype(np.float32),
    }
    out = kernel(**inputs)
    print("out", out.shape, out.dtype, "allones:", bool(np.all(out == 1.0)))
